# revision 12
# baseline (speedup 1.0000x reference)
"""3-layer GAT (DGL GATConv style) on 8 Trainium2 NeuronCores.

Sharding: nodes are padded to NPAD and partitioned into 8 contiguous,
tile-aligned ranges (one per core).  Each core owns the destination
segments of its node range and processes the in-edges of those nodes.
Per layer:
  1. table pass: each core computes ft (features after W) and er for its
     own shard from the transposed hidden state (hT),
  2. AllGather of the bf16 ft table across cores,
  3. edge pass: per-edge dma_gather of ft[src] (bf16) and er[dst]
     (replicated f32 rows), el[src] recomputed on-chip from the gathered
     ft (el = ft . al), max-free edge softmax, and segment-sum
     aggregation via one-hot matmuls accumulated in PSUM.

Edges are bucketed by (dst tile, src chunk) on the host; each bucket is
padded to a fixed number of 128-edge groups so that the SPMD program is
identical across cores (only input data differs).

The gather tile pool is 4 deep (gbufs=4): with the default 2 the
per-edge dma_gathers are starved (only 2 in flight across the 4 SWDGE
queues) and run back-to-back with no overlap against compute, which
costs ~3.7 ms of the ~9.6 ms total.  single_packet=True on dma_gather
wedges the runtime - keep it False.
"""

import numpy as np

import concourse.bass as bass
import concourse.bacc as bacc
import concourse.mybir as mybir
import concourse.tile as tile
from concourse.bass_utils import run_bass_kernel_spmd
from concourse.masks import make_identity

F32 = mybir.dt.float32
BF16 = mybir.dt.bfloat16
I16 = mybir.dt.int16

P = 128


class Cfg:
    """Geometry of the sharded GAT kernel."""

    def __init__(self, n_real, e_real, tiles_per_core=98, tb=7, ncores=8,
                 fin=128, heads=4, dh=32, out=64, neg=0.2):
        self.N = n_real
        self.E = e_real
        self.NCORES = ncores
        self.TILES = tiles_per_core          # node tiles per core
        self.SHARD = tiles_per_core * P      # nodes per core
        self.NPAD = ncores * self.SHARD      # padded node count
        self.CHUNKS = 4
        assert self.NPAD % self.CHUNKS == 0
        self.CR = self.NPAD // self.CHUNKS   # rows per src chunk
        assert self.CR <= 32768 and self.SHARD <= 32768  # int16 gather idx
        self.TB = tb                         # tiles per batch
        assert tiles_per_core % tb == 0
        self.NB = tiles_per_core // tb       # batches
        self.FIN = fin
        self.HEADS = heads
        self.DH = dh
        self.OUT = out
        self.NEG = neg
        assert fin == heads * dh == P
        self.G = None                        # slot groups per (tile, chunk)


# ----------------------------------------------------------------------------
# Host-side preprocessing
# ----------------------------------------------------------------------------

def _wrap_idx(vals, nidx):
    """int array [nidx] -> [128, nidx//16] int16, idx i at [i%16, i//16],
    replicated into all 8 16-partition bands (one per GPSIMD Q7 core)."""
    band = np.asarray(vals, dtype=np.int16).reshape(-1, 16).T
    return np.tile(band, (8, 1))


def prep_edges(cfg, src, dst):
    """Bucket edges by (core, batch, chunk, tile-in-batch); pad buckets to
    a uniform G*128 slots.  Returns the merged per-core meta array
    [NCORES, NB*CHUNKS, P, 2*(NIDX//16)+SLOT] int16 = [ftidx | erridx | dstl]."""
    c = cfg
    src = np.asarray(src, dtype=np.int64)
    dst = np.asarray(dst, dtype=np.int64)

    core = dst // c.SHARD
    tloc = (dst % c.SHARD) // P           # tile within core
    b = tloc // c.TB                      # batch
    ti = tloc % c.TB                      # tile within batch
    chunk = src // c.CR

    nbuck = c.NCORES * c.NB * c.CHUNKS * c.TB
    bucket = ((core * c.NB + b) * c.CHUNKS + chunk) * c.TB + ti
    counts = np.bincount(bucket, minlength=nbuck)
    gmax = int(counts.max())
    G = (gmax + P - 1) // P
    c.G = G
    SLOT = c.TB * G                       # 128-edge groups per (batch, chunk)
    NIDX = SLOT * P                       # gather indices per (batch, chunk)
    cap = G * P

    # sort by bucket, then by src within each bucket: consecutive gather
    # descriptors hit ascending HBM addresses (better row locality)
    order = np.lexsort((src, bucket))
    sorted_bucket = bucket[order]
    boundaries = np.concatenate([[0], np.cumsum(counts)])
    within = np.arange(len(src)) - boundaries[sorted_bucket]

    sb = sorted_bucket
    s_core = sb // (c.NB * c.CHUNKS * c.TB)
    rem = sb % (c.NB * c.CHUNKS * c.TB)
    s_b = rem // (c.CHUNKS * c.TB)
    rem = rem % (c.CHUNKS * c.TB)
    s_chunk = rem // c.TB
    s_ti = rem % c.TB
    flat = ((s_b * c.CHUNKS + s_chunk) * c.TB + s_ti) * cap + within

    src_s = src[order]
    dst_s = dst[order]

    nslots = c.NB * c.CHUNKS * c.TB * cap
    per_core_ft = np.zeros((c.NCORES, nslots), np.int16)
    per_core_er = np.zeros_like(per_core_ft)
    per_core_dl = np.full((c.NCORES, nslots), -1, np.int16)
    for k in range(c.NCORES):
        m = s_core == k
        per_core_ft[k, flat[m]] = (src_s[m] % c.CR).astype(np.int16)
        per_core_er[k, flat[m]] = (dst_s[m] % c.SHARD).astype(np.int16)
        per_core_dl[k, flat[m]] = (dst_s[m] % P).astype(np.int16)

    W16 = NIDX // 16
    meta = np.zeros((c.NCORES, c.NB * c.CHUNKS, P, 2 * W16 + SLOT), np.int16)
    v_ft = per_core_ft.reshape(c.NCORES, c.NB, c.CHUNKS, NIDX)
    v_er = per_core_er.reshape(c.NCORES, c.NB, c.CHUNKS, NIDX)
    v_dl = per_core_dl.reshape(c.NCORES, c.NB, c.CHUNKS, SLOT, P)
    for k in range(c.NCORES):
        for bb in range(c.NB):
            for ch in range(c.CHUNKS):
                bc = bb * c.CHUNKS + ch
                meta[k, bc, :, 0:W16] = _wrap_idx(v_ft[k, bb, ch], NIDX)
                meta[k, bc, :, W16:2 * W16] = _wrap_idx(v_er[k, bb, ch], NIDX)
                meta[k, bc, :, 2 * W16:] = v_dl[k, bb, ch].T
    return meta


def prep_weights(cfg, W, al, ar):
    """[W | Wl | Wr] with Wl[f,h] = sum_d W[f, h*D+d]*al[h,d]."""
    H, D = al.shape
    Wv = W.reshape(W.shape[0], H, D)
    Wl = np.einsum("fhd,hd->fh", Wv, al)
    Wr = np.einsum("fhd,hd->fh", Wv, ar)
    return np.concatenate([W, Wl, Wr], axis=1).astype(np.float32)


# ----------------------------------------------------------------------------
# Device program
# ----------------------------------------------------------------------------

def build_program(cfg, bench_compute=0, bench_ag=0, no_cc=False, one_queue=False,
                  no_gather=False, sp=False, gbufs=4, ebufs=2, ibufs=3,
                  qrot=False, sgp=False):
    c = cfg
    G = c.G
    SLOT = c.TB * G
    NIDX = SLOT * P
    W16 = NIDX // 16
    H0, D0, F0 = c.HEADS, c.DH, P     # layers 0/1
    H2, D2, F2 = 1, c.OUT, c.OUT      # layer 2

    nc = bacc.Bacc("TRN2", target_bir_lowering=False, debug=False,
                   num_devices=c.NCORES, num_swdge_queues=4,
                   dynamic_dma_scratch_size=32768)

    # ---- I/O ----
    featT_own = nc.dram_tensor("featT_own", [P, c.SHARD], F32, kind="ExternalInput")
    meta_in = nc.dram_tensor("meta", [c.NB * c.CHUNKS, P, 2 * W16 + SLOT], I16,
                             kind="ExternalInput")
    wc0_in = nc.dram_tensor("wc0", [P, F0 + 2 * H0], F32, kind="ExternalInput")
    wc1_in = nc.dram_tensor("wc1", [P, F0 + 2 * H0], F32, kind="ExternalInput")
    wc2_in = nc.dram_tensor("wc2", [P, F2 + 2 * H2], F32, kind="ExternalInput")
    bias0_in = nc.dram_tensor("bias0", [P, F0], F32, kind="ExternalInput")
    bias1_in = nc.dram_tensor("bias1", [P, F0], F32, kind="ExternalInput")
    bias2_in = nc.dram_tensor("bias2", [P, F2], F32, kind="ExternalInput")
    alf0_in = nc.dram_tensor("alf0", [P, F0], F32, kind="ExternalInput")
    alf1_in = nc.dram_tensor("alf1", [P, F0], F32, kind="ExternalInput")
    alf2_in = nc.dram_tensor("alf2", [P, F2], F32, kind="ExternalInput")
    out_ext = nc.dram_tensor("out_shard", [c.SHARD, F2], F32, kind="ExternalOutput")

    # ---- internal DRAM ----
    def dram(name, shape, dt, shared=False):
        return nc.dram_tensor(name, shape, dt,
                              addr_space="Shared" if shared else "Local")

    FT = [dram("FT0", [c.NPAD, F0], BF16, True),
          dram("FT1", [c.NPAD, F0], BF16, True),
          dram("FT2", [c.NPAD, F2], F32, True)]
    ERR = [dram("ERR0", [c.SHARD, 64], F32),
           dram("ERR1", [c.SHARD, 64], F32),
           dram("ERR2", [c.SHARD, 64], F32)]
    FTS = [dram("FTS0", [c.SHARD, F0], BF16),
           dram("FTS1", [c.SHARD, F0], BF16),
           dram("FTS2", [c.SHARD, F2], F32)]
    HT = [None,
          dram("HT1", [P, c.SHARD], F32),
          dram("HT2", [P, c.SHARD], F32)]

    groups = [list(range(c.NCORES))]

    with tile.TileContext(nc) as tc:
        with (
            tc.tile_pool(name="const", bufs=1) as constp,
            tc.tile_pool(name="tbl", bufs=3) as tblp,
            tc.tile_pool(name="idx", bufs=ibufs) as idxp,
            tc.tile_pool(name="gath", bufs=gbufs) as gathp,
            tc.tile_pool(name="edge", bufs=ebufs) as edgep,
            tc.tile_pool(name="epi", bufs=2) as epip,
            tc.tile_pool(name="psum", bufs=1, space="PSUM") as psump,
        ):
            # ---- constants ----
            ident = constp.tile([P, P], F32, tag="ident")
            make_identity(nc, ident[:])
            iota16 = constp.tile([P, P], I16, tag="iota16")
            nc.gpsimd.iota(iota16[:], pattern=[[1, P]], base=0,
                           channel_multiplier=0)
            wc_sb = [constp.tile([P, F0 + 2 * H0], F32, tag="wc0", name="wc0s"),
                     constp.tile([P, F0 + 2 * H0], F32, tag="wc1", name="wc1s"),
                     constp.tile([P, F2 + 2 * H2], F32, tag="wc2", name="wc2s")]
            nc.sync.dma_start(wc_sb[0][:], wc0_in[:, :])
            nc.sync.dma_start(wc_sb[1][:], wc1_in[:, :])
            nc.sync.dma_start(wc_sb[2][:], wc2_in[:, :])
            bias_sb = [constp.tile([P, F0], F32, tag="b0", name="b0s"),
                       constp.tile([P, F0], F32, tag="b1", name="b1s"),
                       constp.tile([P, F2], F32, tag="b2", name="b2s")]
            nc.sync.dma_start(bias_sb[0][:], bias0_in[:, :])
            nc.sync.dma_start(bias_sb[1][:], bias1_in[:, :])
            nc.sync.dma_start(bias_sb[2][:], bias2_in[:, :])
            alf_sb = [constp.tile([P, F0], F32, tag="al0", name="al0s"),
                      constp.tile([P, F0], F32, tag="al1", name="al1s"),
                      constp.tile([P, F2], F32, tag="al2", name="al2s")]
            nc.sync.dma_start(alf_sb[0][:], alf0_in[:, :])
            nc.sync.dma_start(alf_sb[1][:], alf1_in[:, :])
            nc.sync.dma_start(alf_sb[2][:], alf2_in[:, :])
            nog = {}
            if no_gather:
                nog["ftb_01"] = constp.tile([P, SLOT * F0], BF16, tag="nogf01",
                                            name="nogf01")
                nog["ftb_2"] = constp.tile([P, SLOT * F2], F32, tag="nogf2",
                                           name="nogf2")
                nog["errb"] = constp.tile([P, SLOT * 64], F32, tag="nogerr",
                                          name="nogerr")
                for v in nog.values():
                    nc.gpsimd.memset(v[:], 0.25)

            def table_pass(lyr, h_src):
                """ft/er for own shard from hT (h_src: DRAM [P, SHARD])."""
                F = F2 if lyr == 2 else F0
                H = H2 if lyr == 2 else H0
                ftdt = F32 if lyr == 2 else BF16
                rep = 64 // H
                for t in range(c.TILES):
                    ht = tblp.tile([P, P], F32, tag="ht_in")
                    nc.sync.dma_start(ht[:], h_src[:, bass.ts(t, P)])
                    ps = psump.tile([P, F + 2 * H], F32, tag="agg0")
                    nc.tensor.matmul(ps[:], lhsT=ht[:], rhs=wc_sb[lyr][:, :],
                                     start=True, stop=True)
                    ft_sb = tblp.tile([P, F], ftdt, tag="ft_sb")
                    nc.vector.tensor_copy(ft_sb[:], ps[:, 0:F])
                    er_sb = tblp.tile([P, 64], F32, tag="er_sb")
                    src_ap = ps[:, F + H:F + 2 * H].unsqueeze(1).to_broadcast(
                        [P, rep, H])
                    nc.vector.tensor_copy(
                        er_sb[:].rearrange("p (r h) -> p r h", h=H), src_ap)
                    nc.scalar.dma_start(FTS[lyr][bass.ts(t, P), :], ft_sb[:])
                    nc.scalar.dma_start(ERR[lyr][bass.ts(t, P), :], er_sb[:])

            def ag_tables(lyr):
                if no_cc:
                    nc.sync.dma_start(FT[lyr][0:c.SHARD, :], FTS[lyr][:, :])
                else:
                    nc.gpsimd.collective_compute(
                        "AllGather", mybir.AluOpType.bypass,
                        replica_groups=groups,
                        ins=[FTS[lyr][:, :]], outs=[FT[lyr][:, :]])

            def edge_pass(lyr):
                F = F2 if lyr == 2 else F0
                H = H2 if lyr == 2 else H0
                D = D2 if lyr == 2 else D0
                ftdt = F32 if lyr == 2 else BF16
                relu = lyr != 2
                Q = F + H
                for b in range(c.NB):
                    psums = [psump.tile([P, Q], F32, tag=f"agg{ti}",
                                        name=f"agg{ti}")
                             for ti in range(c.TB)]
                    for ch in range(c.CHUNKS):
                        bc = b * c.CHUNKS + ch
                        meta = idxp.tile([P, 2 * W16 + SLOT], I16, tag="meta")
                        nc.sync.dma_start(meta[:], meta_in[bc, :, :])

                        if no_gather:
                            ftb = nog["ftb_2"] if lyr == 2 else nog["ftb_01"]
                            errb = nog["errb"]
                        else:
                            ftb = gathp.tile([P, SLOT * F], ftdt, tag="ftb")
                            qf = bc % 4 if qrot else (2 * bc) % 4
                            qe = (bc + 1) % 4 if qrot else (2 * bc + 1) % 4
                            nc.gpsimd.dma_gather(
                                ftb[:].rearrange("p (s f) -> p s f", f=F),
                                FT[lyr][bass.ds(ch * c.CR, c.CR), :],
                                meta[:, 0:W16], NIDX, NIDX, F,
                                single_packet=sp,
                                queue_num=0 if one_queue else qf)
                            errb = gathp.tile([P, SLOT * 64], F32, tag="errb")
                            nc.gpsimd.dma_gather(
                                errb[:].rearrange("p (s f) -> p s f", f=64),
                                ERR[lyr][:, :],
                                meta[:, W16:2 * W16], NIDX, NIDX, 64,
                                single_packet=sp,
                                queue_num=0 if one_queue else qe)

                        # el = ft . al (per edge, from gathered bf16 ft)
                        tmp = edgep.tile([P, SLOT * F], BF16, tag="tmp")
                        nc.vector.tensor_tensor(
                            tmp[:].rearrange("p (s f) -> p s f", f=F),
                            ftb[:].rearrange("p (s f) -> p s f", f=F),
                            alf_sb[lyr][:].unsqueeze(1).to_broadcast(
                                [P, SLOT, F]),
                            op=mybir.AluOpType.mult)
                        el = edgep.tile([P, SLOT * H], F32, tag="el")
                        nc.vector.tensor_reduce(
                            el[:].rearrange("p (s h) -> p s h", h=H),
                            tmp[:].rearrange("p (s h d) -> p s h d", h=H, d=D),
                            axis=mybir.AxisListType.X, op=mybir.AluOpType.add)

                        x = edgep.tile([P, SLOT * H], F32, tag="x")
                        nc.vector.tensor_tensor(
                            x[:].rearrange("p (s h) -> p s h", h=H),
                            el[:].rearrange("p (s h) -> p s h", h=H),
                            errb[:].rearrange("p (s f) -> p s f", f=64)[:, :, 0:H],
                            op=mybir.AluOpType.add)
                        x2 = edgep.tile([P, SLOT * H], F32, tag="x2")
                        nc.vector.scalar_tensor_tensor(
                            x2[:], in0=x[:], scalar=c.NEG, in1=x[:],
                            op0=mybir.AluOpType.mult, op1=mybir.AluOpType.max)
                        exb = edgep.tile([P, SLOT * H], BF16, tag="exb")
                        nc.scalar.activation(exb[:], x2[:],
                                             mybir.ActivationFunctionType.Exp)

                        S = edgep.tile([P, SLOT * P], BF16, tag="S")
                        s_eng = nc.gpsimd if sgp else nc.vector
                        s_eng.tensor_tensor(
                            S[:].rearrange("p (s n) -> p s n", n=P),
                            meta[:, 2 * W16:].unsqueeze(2).to_broadcast(
                                [P, SLOT, P]),
                            iota16[:].unsqueeze(1).to_broadcast([P, SLOT, P]),
                            op=mybir.AluOpType.is_equal)

                        msgx = edgep.tile([P, SLOT * Q], BF16, tag="msgx")
                        mv = msgx[:].rearrange("p (s q) -> p s q", q=Q)
                        nc.vector.tensor_tensor(
                            mv[:, :, 0:F].rearrange("p s (h d) -> p s h d", d=D),
                            ftb[:].rearrange("p (s h d) -> p s h d", h=H, d=D),
                            exb[:].rearrange("p (s h) -> p s h", h=H)
                                .unsqueeze(3).to_broadcast([P, SLOT, H, D]),
                            op=mybir.AluOpType.mult)
                        nc.scalar.activation(
                            mv[:, :, F:Q],
                            exb[:].rearrange("p (s h) -> p s h", h=H),
                            mybir.ActivationFunctionType.Copy)

                        for ti in range(c.TB):
                            for g in range(G):
                                s = ti * G + g
                                nc.tensor.matmul(
                                    psums[ti][:, :],
                                    lhsT=S[:, bass.ts(s, P)],
                                    rhs=mv[:, s, :],
                                    start=(ch == 0 and g == 0),
                                    stop=(ch == c.CHUNKS - 1 and g == G - 1))
                    # epilogue
                    for ti in range(c.TB):
                        t = b * c.TB + ti
                        rec = epip.tile([P, H], F32, tag="rec")
                        nc.vector.reciprocal(rec[:], psums[ti][:, F:Q])
                        o = epip.tile([P, F], F32, tag="o")
                        nc.vector.tensor_tensor(
                            o[:].rearrange("p (h d) -> p h d", d=D),
                            psums[ti][:, 0:F].rearrange("p (h d) -> p h d", d=D),
                            rec[:].unsqueeze(2).to_broadcast([P, H, D]),
                            op=mybir.AluOpType.mult)
                        o2 = epip.tile([P, F], F32, tag="o2")
                        nc.vector.tensor_tensor(o2[:], o[:], bias_sb[lyr][:, :],
                                                op=mybir.AluOpType.add)
                        if relu:
                            o3 = epip.tile([P, F], F32, tag="o3")
                            nc.scalar.activation(
                                o3[:], o2[:], mybir.ActivationFunctionType.Relu)
                            pst = psump.tile([P, P], F32, tag="ptr")
                            nc.tensor.transpose(pst[:], o3[:], ident[:])
                            htile = epip.tile([P, P], F32, tag="htile")
                            nc.scalar.activation(htile[:], pst[:],
                                                 mybir.ActivationFunctionType.Copy)
                            nc.scalar.dma_start(HT[lyr + 1][:, bass.ts(t, P)],
                                                htile[:])
                        else:
                            nc.scalar.dma_start(out_ext[bass.ts(t, P), :], o2[:])

            if bench_compute:
                for lyr in range(3):
                    ag_tables(lyr)

                def compute_body(_i):
                    table_pass(0, featT_own)
                    edge_pass(0)
                    table_pass(1, HT[1])
                    edge_pass(1)
                    table_pass(2, HT[2])
                    edge_pass(2)
                with tc.For_i(0, bench_compute, 1) as i:
                    compute_body(i)
            elif bench_ag:
                table_pass(0, featT_own)
                table_pass(1, featT_own)
                table_pass(2, featT_own)
                for _ in range(bench_ag):
                    for lyr in range(3):
                        ag_tables(lyr)
            else:
                table_pass(0, featT_own)
                ag_tables(0)
                edge_pass(0)
                table_pass(1, HT[1])
                ag_tables(1)
                edge_pass(1)
                table_pass(2, HT[2])
                ag_tables(2)
                edge_pass(2)

    nc.compile()
    return nc


# ----------------------------------------------------------------------------
# Host entry points
# ----------------------------------------------------------------------------

def make_in_maps(cfg, features, src, dst, weights):
    """weights: dict with W0,al0,ar0,b0,W1,...  Returns list of in_maps."""
    c = cfg
    meta = prep_edges(c, src, dst)
    wc0 = prep_weights(c, weights["W0"], weights["al0"], weights["ar0"])
    wc1 = prep_weights(c, weights["W1"], weights["al1"], weights["ar1"])
    wc2 = prep_weights(c, weights["W2"], weights["al2"], weights["ar2"])
    b0 = np.tile(np.asarray(weights["b0"], np.float32), (P, 1))
    b1 = np.tile(np.asarray(weights["b1"], np.float32), (P, 1))
    b2 = np.tile(np.asarray(weights["b2"], np.float32), (P, 1))
    al0 = np.tile(np.asarray(weights["al0"], np.float32).reshape(-1), (P, 1))
    al1 = np.tile(np.asarray(weights["al1"], np.float32).reshape(-1), (P, 1))
    al2 = np.tile(np.asarray(weights["al2"], np.float32).reshape(-1), (P, 1))

    featpadT = np.zeros((P, c.NPAD), np.float32)
    featpadT[:, :c.N] = np.asarray(features, np.float32).T

    in_maps = []
    for k in range(c.NCORES):
        in_maps.append({
            "featT_own": np.ascontiguousarray(
                featpadT[:, k * c.SHARD:(k + 1) * c.SHARD]),
            "meta": meta[k],
            "wc0": wc0, "wc1": wc1, "wc2": wc2,
            "bias0": b0, "bias1": b1, "bias2": b2,
            "alf0": al0, "alf1": al1, "alf2": al2,
        })
    return in_maps


def unshard_output(cfg, results):
    c = cfg
    parts = [results[k]["out_shard"] for k in range(c.NCORES)]
    return np.concatenate(parts, axis=0)[:c.N].astype(np.float32)


def kernel(features, src, dst, W0, al0, ar0, b0, W1, al1, ar1, b1,
           W2, al2, ar2, b2):
    cfg = Cfg(100000, 1600000)
    weights = dict(W0=np.asarray(W0), al0=np.asarray(al0), ar0=np.asarray(ar0),
                   b0=np.asarray(b0), W1=np.asarray(W1), al1=np.asarray(al1),
                   ar1=np.asarray(ar1), b1=np.asarray(b1), W2=np.asarray(W2),
                   al2=np.asarray(al2), ar2=np.asarray(ar2), b2=np.asarray(b2))
    in_maps = make_in_maps(cfg, np.asarray(features), np.asarray(src),
                           np.asarray(dst), weights)
    nc = build_program(cfg)
    res = run_bass_kernel_spmd(nc, in_maps, list(range(cfg.NCORES)))
    return unshard_output(cfg, res.results)



# revision 13
# speedup vs baseline: 1.1076x; 1.1076x over previous
"""3-layer GAT (DGL GATConv style) on 8 Trainium2 NeuronCores.

Sharding: nodes are padded to NPAD and partitioned into 8 contiguous,
tile-aligned ranges (one per core).  Each core owns the destination
segments of its node range and processes the in-edges of those nodes.
Per layer:
  1. table pass: each core computes ft (features after W) and er for its
     own shard from the transposed hidden state (hT),
  2. AllGather of the bf16 ft table across cores,
  3. edge pass: per-edge dma_gather of ft[src] (bf16) and er[dst]
     (replicated f32 rows), el[src] recomputed on-chip from the gathered
     ft (el = ft . al), max-free edge softmax, and segment-sum
     aggregation via one-hot matmuls accumulated in PSUM.

Edges are bucketed by (dst tile, src chunk) on the host; each bucket is
padded to a fixed number of 128-edge groups so that the SPMD program is
identical across cores (only input data differs).

The gather tile pool is 4 deep (gbufs=4): with the default 2 the
per-edge dma_gathers are starved (only 2 in flight across the 4 SWDGE
queues) and run back-to-back with no overlap against compute, which
costs ~3.7 ms of the ~9.6 ms total.  single_packet=True on dma_gather
wedges the runtime - keep it False.
"""

import numpy as np

import concourse.bass as bass
import concourse.bacc as bacc
import concourse.mybir as mybir
import concourse.tile as tile
from concourse.bass_utils import run_bass_kernel_spmd
from concourse.masks import make_identity

F32 = mybir.dt.float32
BF16 = mybir.dt.bfloat16
I16 = mybir.dt.int16

P = 128


class Cfg:
    """Geometry of the sharded GAT kernel."""

    def __init__(self, n_real, e_real, tiles_per_core=98, tb=7, ncores=8,
                 fin=128, heads=4, dh=32, out=64, neg=0.2):
        self.N = n_real
        self.E = e_real
        self.NCORES = ncores
        self.TILES = tiles_per_core          # node tiles per core
        self.SHARD = tiles_per_core * P      # nodes per core
        self.NPAD = ncores * self.SHARD      # padded node count
        self.CHUNKS = 4
        assert self.NPAD % self.CHUNKS == 0
        self.CR = self.NPAD // self.CHUNKS   # rows per src chunk
        assert self.CR <= 32768 and self.SHARD <= 32768  # int16 gather idx
        self.TB = tb                         # tiles per batch
        assert tiles_per_core % tb == 0
        self.NB = tiles_per_core // tb       # batches
        self.FIN = fin
        self.HEADS = heads
        self.DH = dh
        self.OUT = out
        self.NEG = neg
        assert fin == heads * dh == P
        self.G = None                        # slot groups per (tile, chunk)


# ----------------------------------------------------------------------------
# Host-side preprocessing
# ----------------------------------------------------------------------------

def _wrap_idx(vals, nidx):
    """int array [nidx] -> [128, nidx//16] int16, idx i at [i%16, i//16],
    replicated into all 8 16-partition bands (one per GPSIMD Q7 core)."""
    band = np.asarray(vals, dtype=np.int16).reshape(-1, 16).T
    return np.tile(band, (8, 1))


def prep_edges(cfg, src, dst):
    """Bucket edges by (core, batch, chunk, tile-in-batch); pad buckets to
    a uniform G*128 slots.  Returns the merged per-core meta array
    [NCORES, NB*CHUNKS, P, 2*(NIDX//16)+SLOT] int16 = [ftidx | erridx | dstl]."""
    c = cfg
    src = np.asarray(src, dtype=np.int64)
    dst = np.asarray(dst, dtype=np.int64)

    core = dst // c.SHARD
    tloc = (dst % c.SHARD) // P           # tile within core
    b = tloc // c.TB                      # batch
    ti = tloc % c.TB                      # tile within batch
    chunk = src // c.CR

    nbuck = c.NCORES * c.NB * c.CHUNKS * c.TB
    bucket = ((core * c.NB + b) * c.CHUNKS + chunk) * c.TB + ti
    counts = np.bincount(bucket, minlength=nbuck)
    gmax = int(counts.max())
    G = (gmax + P - 1) // P
    c.G = G
    SLOT = c.TB * G                       # 128-edge groups per (batch, chunk)
    NIDX = SLOT * P                       # gather indices per (batch, chunk)
    cap = G * P

    # sort by bucket, then by src within each bucket: consecutive gather
    # descriptors hit ascending HBM addresses (better row locality)
    order = np.lexsort((src, bucket))
    sorted_bucket = bucket[order]
    boundaries = np.concatenate([[0], np.cumsum(counts)])
    within = np.arange(len(src)) - boundaries[sorted_bucket]

    sb = sorted_bucket
    s_core = sb // (c.NB * c.CHUNKS * c.TB)
    rem = sb % (c.NB * c.CHUNKS * c.TB)
    s_b = rem // (c.CHUNKS * c.TB)
    rem = rem % (c.CHUNKS * c.TB)
    s_chunk = rem // c.TB
    s_ti = rem % c.TB
    flat = ((s_b * c.CHUNKS + s_chunk) * c.TB + s_ti) * cap + within

    src_s = src[order]
    dst_s = dst[order]

    nslots = c.NB * c.CHUNKS * c.TB * cap
    per_core_ft = np.zeros((c.NCORES, nslots), np.int16)
    per_core_er = np.zeros_like(per_core_ft)
    per_core_dl = np.full((c.NCORES, nslots), -1, np.int16)
    for k in range(c.NCORES):
        m = s_core == k
        per_core_ft[k, flat[m]] = (src_s[m] % c.CR).astype(np.int16)
        per_core_er[k, flat[m]] = (dst_s[m] % c.SHARD).astype(np.int16)
        per_core_dl[k, flat[m]] = (dst_s[m] % P).astype(np.int16)

    W16 = NIDX // 16
    meta = np.zeros((c.NCORES, c.NB * c.CHUNKS, P, 2 * W16 + SLOT), np.int16)
    v_ft = per_core_ft.reshape(c.NCORES, c.NB, c.CHUNKS, NIDX)
    v_er = per_core_er.reshape(c.NCORES, c.NB, c.CHUNKS, NIDX)
    v_dl = per_core_dl.reshape(c.NCORES, c.NB, c.CHUNKS, SLOT, P)
    for k in range(c.NCORES):
        for bb in range(c.NB):
            for ch in range(c.CHUNKS):
                bc = bb * c.CHUNKS + ch
                meta[k, bc, :, 0:W16] = _wrap_idx(v_ft[k, bb, ch], NIDX)
                meta[k, bc, :, W16:2 * W16] = _wrap_idx(v_er[k, bb, ch], NIDX)
                meta[k, bc, :, 2 * W16:] = v_dl[k, bb, ch].T
    return meta


def prep_weights(cfg, W, al, ar):
    """[W | Wl | Wr] with Wl[f,h] = sum_d W[f, h*D+d]*al[h,d]."""
    H, D = al.shape
    Wv = W.reshape(W.shape[0], H, D)
    Wl = np.einsum("fhd,hd->fh", Wv, al)
    Wr = np.einsum("fhd,hd->fh", Wv, ar)
    return np.concatenate([W, Wl, Wr], axis=1).astype(np.float32)


# ----------------------------------------------------------------------------
# Device program
# ----------------------------------------------------------------------------

def build_program(cfg, bench_compute=0, bench_ag=0, no_cc=False, one_queue=False,
                  no_gather=False, sp=False, gbufs=4, ebufs=2, ibufs=4,
                  qrot=False, sgp=False):
    c = cfg
    G = c.G
    SLOT = c.TB * G
    NIDX = SLOT * P
    W16 = NIDX // 16
    H0, D0, F0 = c.HEADS, c.DH, P     # layers 0/1
    H2, D2, F2 = 1, c.OUT, c.OUT      # layer 2

    nc = bacc.Bacc("TRN2", target_bir_lowering=False, debug=False,
                   num_devices=c.NCORES, num_swdge_queues=4,
                   dynamic_dma_scratch_size=32768)

    # ---- I/O ----
    featT_own = nc.dram_tensor("featT_own", [P, c.SHARD], F32, kind="ExternalInput")
    meta_in = nc.dram_tensor("meta", [c.NB * c.CHUNKS, P, 2 * W16 + SLOT], I16,
                             kind="ExternalInput")
    wc0_in = nc.dram_tensor("wc0", [P, F0 + 2 * H0], F32, kind="ExternalInput")
    wc1_in = nc.dram_tensor("wc1", [P, F0 + 2 * H0], F32, kind="ExternalInput")
    wc2_in = nc.dram_tensor("wc2", [P, F2 + 2 * H2], F32, kind="ExternalInput")
    bias0_in = nc.dram_tensor("bias0", [P, F0], F32, kind="ExternalInput")
    bias1_in = nc.dram_tensor("bias1", [P, F0], F32, kind="ExternalInput")
    bias2_in = nc.dram_tensor("bias2", [P, F2], F32, kind="ExternalInput")
    alf0_in = nc.dram_tensor("alf0", [P, F0], F32, kind="ExternalInput")
    alf1_in = nc.dram_tensor("alf1", [P, F0], F32, kind="ExternalInput")
    alf2_in = nc.dram_tensor("alf2", [P, F2], F32, kind="ExternalInput")
    out_ext = nc.dram_tensor("out_shard", [c.SHARD, F2], F32, kind="ExternalOutput")

    # ---- internal DRAM ----
    def dram(name, shape, dt, shared=False):
        return nc.dram_tensor(name, shape, dt,
                              addr_space="Shared" if shared else "Local")

    FT = [dram("FT0", [c.NPAD, F0], BF16, True),
          dram("FT1", [c.NPAD, F0], BF16, True),
          dram("FT2", [c.NPAD, F2], F32, True)]
    ERR = [dram("ERR0", [c.SHARD, 64], F32),
           dram("ERR1", [c.SHARD, 64], F32),
           dram("ERR2", [c.SHARD, 64], F32)]
    FTS = [dram("FTS0", [c.SHARD, F0], BF16),
           dram("FTS1", [c.SHARD, F0], BF16),
           dram("FTS2", [c.SHARD, F2], F32)]
    HT = [None,
          dram("HT1", [P, c.SHARD], F32),
          dram("HT2", [P, c.SHARD], F32)]

    groups = [list(range(c.NCORES))]

    with tile.TileContext(nc) as tc:
        with (
            tc.tile_pool(name="const", bufs=1) as constp,
            tc.tile_pool(name="tbl", bufs=3) as tblp,
            tc.tile_pool(name="idx", bufs=ibufs) as idxp,
            tc.tile_pool(name="gath", bufs=gbufs) as gathp,
            tc.tile_pool(name="edge", bufs=ebufs) as edgep,
            tc.tile_pool(name="epi", bufs=2) as epip,
            tc.tile_pool(name="psum", bufs=1, space="PSUM") as psump,
        ):
            # ---- constants ----
            ident = constp.tile([P, P], F32, tag="ident")
            make_identity(nc, ident[:])
            iota16 = constp.tile([P, P], I16, tag="iota16")
            nc.gpsimd.iota(iota16[:], pattern=[[1, P]], base=0,
                           channel_multiplier=0)
            wc_sb = [constp.tile([P, F0 + 2 * H0], F32, tag="wc0", name="wc0s"),
                     constp.tile([P, F0 + 2 * H0], F32, tag="wc1", name="wc1s"),
                     constp.tile([P, F2 + 2 * H2], F32, tag="wc2", name="wc2s")]
            nc.sync.dma_start(wc_sb[0][:], wc0_in[:, :])
            nc.sync.dma_start(wc_sb[1][:], wc1_in[:, :])
            nc.sync.dma_start(wc_sb[2][:], wc2_in[:, :])
            bias_sb = [constp.tile([P, F0], F32, tag="b0", name="b0s"),
                       constp.tile([P, F0], F32, tag="b1", name="b1s"),
                       constp.tile([P, F2], F32, tag="b2", name="b2s")]
            nc.sync.dma_start(bias_sb[0][:], bias0_in[:, :])
            nc.sync.dma_start(bias_sb[1][:], bias1_in[:, :])
            nc.sync.dma_start(bias_sb[2][:], bias2_in[:, :])
            alf_sb = [constp.tile([P, F0], F32, tag="al0", name="al0s"),
                      constp.tile([P, F0], F32, tag="al1", name="al1s"),
                      constp.tile([P, F2], F32, tag="al2", name="al2s")]
            nc.sync.dma_start(alf_sb[0][:], alf0_in[:, :])
            nc.sync.dma_start(alf_sb[1][:], alf1_in[:, :])
            nc.sync.dma_start(alf_sb[2][:], alf2_in[:, :])
            nog = {}
            if no_gather:
                nog["ftb_01"] = constp.tile([P, SLOT * F0], BF16, tag="nogf01",
                                            name="nogf01")
                nog["ftb_2"] = constp.tile([P, SLOT * F2], F32, tag="nogf2",
                                           name="nogf2")
                nog["errb"] = constp.tile([P, SLOT * 64], F32, tag="nogerr",
                                          name="nogerr")
                for v in nog.values():
                    nc.gpsimd.memset(v[:], 0.25)

            def table_pass(lyr, h_src):
                """ft/er for own shard from hT (h_src: DRAM [P, SHARD])."""
                F = F2 if lyr == 2 else F0
                H = H2 if lyr == 2 else H0
                ftdt = F32 if lyr == 2 else BF16
                rep = 64 // H
                for t in range(c.TILES):
                    ht = tblp.tile([P, P], F32, tag="ht_in")
                    nc.sync.dma_start(ht[:], h_src[:, bass.ts(t, P)])
                    ps = psump.tile([P, F + 2 * H], F32, tag="agg0")
                    nc.tensor.matmul(ps[:], lhsT=ht[:], rhs=wc_sb[lyr][:, :],
                                     start=True, stop=True)
                    ft_sb = tblp.tile([P, F], ftdt, tag="ft_sb")
                    nc.vector.tensor_copy(ft_sb[:], ps[:, 0:F])
                    er_sb = tblp.tile([P, 64], F32, tag="er_sb")
                    src_ap = ps[:, F + H:F + 2 * H].unsqueeze(1).to_broadcast(
                        [P, rep, H])
                    nc.vector.tensor_copy(
                        er_sb[:].rearrange("p (r h) -> p r h", h=H), src_ap)
                    nc.scalar.dma_start(FTS[lyr][bass.ts(t, P), :], ft_sb[:])
                    nc.scalar.dma_start(ERR[lyr][bass.ts(t, P), :], er_sb[:])

            def ag_tables(lyr):
                if no_cc:
                    nc.sync.dma_start(FT[lyr][0:c.SHARD, :], FTS[lyr][:, :])
                else:
                    nc.gpsimd.collective_compute(
                        "AllGather", mybir.AluOpType.bypass,
                        replica_groups=groups,
                        ins=[FTS[lyr][:, :]], outs=[FT[lyr][:, :]])

            def edge_pass(lyr):
                F = F2 if lyr == 2 else F0
                H = H2 if lyr == 2 else H0
                D = D2 if lyr == 2 else D0
                ftdt = F32 if lyr == 2 else BF16
                relu = lyr != 2
                Q = F + H
                for b in range(c.NB):
                    psums = [psump.tile([P, Q], F32, tag=f"agg{ti}",
                                        name=f"agg{ti}")
                             for ti in range(c.TB)]
                    for ch in range(c.CHUNKS):
                        bc = b * c.CHUNKS + ch
                        meta = idxp.tile([P, 2 * W16 + SLOT], I16, tag="meta")
                        nc.sync.dma_start(meta[:], meta_in[bc, :, :])

                        if no_gather:
                            ftb = nog["ftb_2"] if lyr == 2 else nog["ftb_01"]
                            errb = nog["errb"]
                        else:
                            ftb = gathp.tile([P, SLOT * F], ftdt, tag="ftb")
                            qf = bc % 4 if qrot else (2 * bc) % 4
                            qe = (bc + 1) % 4 if qrot else (2 * bc + 1) % 4
                            nc.gpsimd.dma_gather(
                                ftb[:].rearrange("p (s f) -> p s f", f=F),
                                FT[lyr][bass.ds(ch * c.CR, c.CR), :],
                                meta[:, 0:W16], NIDX, NIDX, F,
                                single_packet=sp,
                                queue_num=0 if one_queue else qf)
                            errb = gathp.tile([P, SLOT * 64], F32, tag="errb")
                            nc.gpsimd.dma_gather(
                                errb[:].rearrange("p (s f) -> p s f", f=64),
                                ERR[lyr][:, :],
                                meta[:, W16:2 * W16], NIDX, NIDX, 64,
                                single_packet=sp,
                                queue_num=0 if one_queue else qe)

                        # el = ft . al (per edge, from gathered bf16 ft)
                        tmp = edgep.tile([P, SLOT * F], BF16, tag="tmp")
                        nc.vector.tensor_tensor(
                            tmp[:].rearrange("p (s f) -> p s f", f=F),
                            ftb[:].rearrange("p (s f) -> p s f", f=F),
                            alf_sb[lyr][:].unsqueeze(1).to_broadcast(
                                [P, SLOT, F]),
                            op=mybir.AluOpType.mult)
                        el = edgep.tile([P, SLOT * H], F32, tag="el")
                        nc.vector.tensor_reduce(
                            el[:].rearrange("p (s h) -> p s h", h=H),
                            tmp[:].rearrange("p (s h d) -> p s h d", h=H, d=D),
                            axis=mybir.AxisListType.X, op=mybir.AluOpType.add)

                        x = edgep.tile([P, SLOT * H], F32, tag="x")
                        nc.vector.tensor_tensor(
                            x[:].rearrange("p (s h) -> p s h", h=H),
                            el[:].rearrange("p (s h) -> p s h", h=H),
                            errb[:].rearrange("p (s f) -> p s f", f=64)[:, :, 0:H],
                            op=mybir.AluOpType.add)
                        x2 = edgep.tile([P, SLOT * H], F32, tag="x2")
                        nc.vector.scalar_tensor_tensor(
                            x2[:], in0=x[:], scalar=c.NEG, in1=x[:],
                            op0=mybir.AluOpType.mult, op1=mybir.AluOpType.max)
                        exb = edgep.tile([P, SLOT * H], BF16, tag="exb")
                        nc.scalar.activation(exb[:], x2[:],
                                             mybir.ActivationFunctionType.Exp)

                        S = edgep.tile([P, SLOT * P], BF16, tag="S")
                        s_eng = nc.gpsimd if sgp else nc.vector
                        s_eng.tensor_tensor(
                            S[:].rearrange("p (s n) -> p s n", n=P),
                            meta[:, 2 * W16:].unsqueeze(2).to_broadcast(
                                [P, SLOT, P]),
                            iota16[:].unsqueeze(1).to_broadcast([P, SLOT, P]),
                            op=mybir.AluOpType.is_equal)

                        msgx = edgep.tile([P, SLOT * Q], BF16, tag="msgx")
                        mv = msgx[:].rearrange("p (s q) -> p s q", q=Q)
                        nc.vector.tensor_tensor(
                            mv[:, :, 0:F].rearrange("p s (h d) -> p s h d", d=D),
                            ftb[:].rearrange("p (s h d) -> p s h d", h=H, d=D),
                            exb[:].rearrange("p (s h) -> p s h", h=H)
                                .unsqueeze(3).to_broadcast([P, SLOT, H, D]),
                            op=mybir.AluOpType.mult)
                        nc.scalar.activation(
                            mv[:, :, F:Q],
                            exb[:].rearrange("p (s h) -> p s h", h=H),
                            mybir.ActivationFunctionType.Copy)

                        for ti in range(c.TB):
                            for g in range(G):
                                s = ti * G + g
                                nc.tensor.matmul(
                                    psums[ti][:, :],
                                    lhsT=S[:, bass.ts(s, P)],
                                    rhs=mv[:, s, :],
                                    start=(ch == 0 and g == 0),
                                    stop=(ch == c.CHUNKS - 1 and g == G - 1))
                    # epilogue
                    for ti in range(c.TB):
                        t = b * c.TB + ti
                        rec = epip.tile([P, H], F32, tag="rec")
                        nc.vector.reciprocal(rec[:], psums[ti][:, F:Q])
                        o = epip.tile([P, F], F32, tag="o")
                        nc.vector.tensor_tensor(
                            o[:].rearrange("p (h d) -> p h d", d=D),
                            psums[ti][:, 0:F].rearrange("p (h d) -> p h d", d=D),
                            rec[:].unsqueeze(2).to_broadcast([P, H, D]),
                            op=mybir.AluOpType.mult)
                        o2 = epip.tile([P, F], F32, tag="o2")
                        nc.vector.tensor_tensor(o2[:], o[:], bias_sb[lyr][:, :],
                                                op=mybir.AluOpType.add)
                        if relu:
                            o3 = epip.tile([P, F], F32, tag="o3")
                            nc.scalar.activation(
                                o3[:], o2[:], mybir.ActivationFunctionType.Relu)
                            pst = psump.tile([P, P], F32, tag="ptr")
                            nc.tensor.transpose(pst[:], o3[:], ident[:])
                            htile = epip.tile([P, P], F32, tag="htile")
                            nc.scalar.activation(htile[:], pst[:],
                                                 mybir.ActivationFunctionType.Copy)
                            nc.scalar.dma_start(HT[lyr + 1][:, bass.ts(t, P)],
                                                htile[:])
                        else:
                            nc.scalar.dma_start(out_ext[bass.ts(t, P), :], o2[:])

            if bench_compute:
                for lyr in range(3):
                    ag_tables(lyr)

                def compute_body(_i):
                    table_pass(0, featT_own)
                    edge_pass(0)
                    table_pass(1, HT[1])
                    edge_pass(1)
                    table_pass(2, HT[2])
                    edge_pass(2)
                with tc.For_i(0, bench_compute, 1) as i:
                    compute_body(i)
            elif bench_ag:
                table_pass(0, featT_own)
                table_pass(1, featT_own)
                table_pass(2, featT_own)
                for _ in range(bench_ag):
                    for lyr in range(3):
                        ag_tables(lyr)
            else:
                table_pass(0, featT_own)
                ag_tables(0)
                edge_pass(0)
                table_pass(1, HT[1])
                ag_tables(1)
                edge_pass(1)
                table_pass(2, HT[2])
                ag_tables(2)
                edge_pass(2)

    nc.compile()
    return nc


# ----------------------------------------------------------------------------
# Host entry points
# ----------------------------------------------------------------------------

def make_in_maps(cfg, features, src, dst, weights):
    """weights: dict with W0,al0,ar0,b0,W1,...  Returns list of in_maps."""
    c = cfg
    meta = prep_edges(c, src, dst)
    wc0 = prep_weights(c, weights["W0"], weights["al0"], weights["ar0"])
    wc1 = prep_weights(c, weights["W1"], weights["al1"], weights["ar1"])
    wc2 = prep_weights(c, weights["W2"], weights["al2"], weights["ar2"])
    b0 = np.tile(np.asarray(weights["b0"], np.float32), (P, 1))
    b1 = np.tile(np.asarray(weights["b1"], np.float32), (P, 1))
    b2 = np.tile(np.asarray(weights["b2"], np.float32), (P, 1))
    al0 = np.tile(np.asarray(weights["al0"], np.float32).reshape(-1), (P, 1))
    al1 = np.tile(np.asarray(weights["al1"], np.float32).reshape(-1), (P, 1))
    al2 = np.tile(np.asarray(weights["al2"], np.float32).reshape(-1), (P, 1))

    featpadT = np.zeros((P, c.NPAD), np.float32)
    featpadT[:, :c.N] = np.asarray(features, np.float32).T

    in_maps = []
    for k in range(c.NCORES):
        in_maps.append({
            "featT_own": np.ascontiguousarray(
                featpadT[:, k * c.SHARD:(k + 1) * c.SHARD]),
            "meta": meta[k],
            "wc0": wc0, "wc1": wc1, "wc2": wc2,
            "bias0": b0, "bias1": b1, "bias2": b2,
            "alf0": al0, "alf1": al1, "alf2": al2,
        })
    return in_maps


def unshard_output(cfg, results):
    c = cfg
    parts = [results[k]["out_shard"] for k in range(c.NCORES)]
    return np.concatenate(parts, axis=0)[:c.N].astype(np.float32)


def kernel(features, src, dst, W0, al0, ar0, b0, W1, al1, ar1, b1,
           W2, al2, ar2, b2):
    cfg = Cfg(100000, 1600000)
    weights = dict(W0=np.asarray(W0), al0=np.asarray(al0), ar0=np.asarray(ar0),
                   b0=np.asarray(b0), W1=np.asarray(W1), al1=np.asarray(al1),
                   ar1=np.asarray(ar1), b1=np.asarray(b1), W2=np.asarray(W2),
                   al2=np.asarray(al2), ar2=np.asarray(ar2), b2=np.asarray(b2))
    in_maps = make_in_maps(cfg, np.asarray(features), np.asarray(src),
                           np.asarray(dst), weights)
    nc = build_program(cfg)
    res = run_bass_kernel_spmd(nc, in_maps, list(range(cfg.NCORES)))
    return unshard_output(cfg, res.results)

